# revision 20
# baseline (speedup 1.0000x reference)
"""CRPS loss kernel for Trainium2, 8 NeuronCores (SPMD data-parallel).

reference semantics:
    p, t = prediction.ravel(), target.ravel()       # N = 16,611,840 each
    lo, hi = min(min p, min t), max(max p, max t)
    x = linspace(lo, hi, 1000)  (f32)
    cdf_q(x_i) = #{v in q : v <= x_i} / N
    return trapz(|cdf_p - cdf_t|^2, x)

Device work (per core, 1/8 shard of each tensor):
  kernel A: min/max reduce (DVE X-reduces + Pool whole-tile max partials).
  kernel B: per element j = rint(v*A + B) in [0, 1000] (Act affine+round);
            digits m = j & 31 (DVE), rh = j >> 5 (Act scaled-round trick).
            Joint (m, rh) histogram via PACK4 block-diagonal PE matmuls:
            lhsT = one-hots of m (4 groups x 32 bins, column order m*4+g),
            rhs  = one-hots of rh (4 groups x 32 bins, order rh*4+g).
            PSUM accumulates the WHOLE tensor (counts < 2^24, exact f32);
            one psum->sbuf copy + DMA per tensor. The device's own binning
            of the pad value (partition 127 is all-pad) is exported as jpad
            so the host subtracts pads exactly.
Host: combine cores' [128, 256] f32 psum dumps -> exact 1024-bin histogram,
      fold j>=999, cumsum, 1000-point trapz in f64.
"""

import numpy as np
from concourse import bacc, mybir, tile
from concourse.bass_utils import run_bass_kernel_spmd

P = 128
NCORES = 8
TOTAL = 16 * 1 * 721 * 1440          # 16,611,840
SHARD = TOTAL // NCORES              # 2,076,480
KTOT = 16640                         # padded columns/core/tensor
PADN = P * KTOT - SHARD              # 53,440
NX = 1000
C = 640                              # chunk columns
NCHUNK = KTOT // C                   # 26 per tensor
NI = C // 4                          # PACK4 matmuls per chunk
RED = KTOT // 4                      # minmax reduce chunk

F32 = mybir.dt.float32
I32 = mybir.dt.int32
BF16 = mybir.dt.bfloat16
ALU = mybir.AluOpType
ACT = mybir.ActivationFunctionType

N_ACT_PLAIN = 7      # max m-side bins built on Act (alternates 7/6)


def _build_minmax():
    nc = bacc.Bacc()
    ins = [
        nc.declare_dram_parameter("pv", [P, KTOT], F32, isOutput=False),
        nc.declare_dram_parameter("tv", [P, KTOT], F32, isOutput=False),
    ]
    out = nc.declare_dram_parameter("mm", [1, 2], F32, isOutput=True)

    with tile.TileContext(nc) as tc:
        with (
            tc.tile_pool(name="sbuf", bufs=6) as pool,
            tc.tile_pool(name="acc", bufs=1) as apool,
        ):
            ntile = 2 * (KTOT // RED)            # 8 tiles
            NPOOL = 5                            # tiles whose MAX runs on Pool
            mins = apool.tile([P, ntile], F32)
            maxs = apool.tile([P, ntile - NPOOL], F32)
            pmax1 = apool.tile([1, NPOOL], F32)
            t = 0
            for src in ins:
                for ci in range(KTOT // RED):
                    v = pool.tile([P, RED], F32, tag="v")
                    dmaeng = nc.sync if t % 2 == 0 else nc.scalar
                    dmaeng.dma_start(v[:], src[:, ci * RED:(ci + 1) * RED])
                    nc.vector.tensor_reduce(
                        mins[:, t:t + 1], v[:], mybir.AxisListType.X, ALU.min)
                    if t < NPOOL:
                        nc.gpsimd.tensor_reduce(
                            pmax1[:, t:t + 1], v[:], mybir.AxisListType.XYZWC,
                            ALU.max)
                    else:
                        nc.vector.tensor_reduce(
                            maxs[:, t - NPOOL:t - NPOOL + 1], v[:],
                            mybir.AxisListType.X, ALU.max)
                    t += 1
            pmin = apool.tile([P, 1], F32)
            pmax = apool.tile([P, 1], F32)
            nc.vector.tensor_reduce(pmin[:], mins[:], mybir.AxisListType.X,
                                    ALU.min)
            nc.vector.tensor_reduce(pmax[:], maxs[:], mybir.AxisListType.X,
                                    ALU.max)
            both = apool.tile([P, 2], F32)
            nc.vector.tensor_scalar(out=both[:, 0:1], in0=pmin[:], scalar1=-1.0,
                                    scalar2=None, op0=ALU.mult)
            nc.vector.tensor_copy(out=both[:, 1:2], in_=pmax[:])
            red = apool.tile([1, 2], F32)
            nc.gpsimd.tensor_reduce(red[:], both[:], mybir.AxisListType.C,
                                    ALU.max)
            pb = apool.tile([1, 1], F32)
            nc.vector.tensor_reduce(pb[:], pmax1[:], mybir.AxisListType.X,
                                    ALU.max)
            fin = apool.tile([1, 2], F32)
            nc.vector.tensor_copy(out=fin[:], in_=red[:])
            nc.vector.tensor_tensor(out=fin[:, 1:2], in0=red[:, 1:2],
                                    in1=pb[:], op=ALU.max)
            nc.sync.dma_start(out[:], fin[:])
    nc.compile()
    return nc


def _build_hist():
    nc = bacc.Bacc()
    ins = [
        nc.declare_dram_parameter("pv", [P, KTOT], F32, isOutput=False),
        nc.declare_dram_parameter("tv", [P, KTOT], F32, isOutput=False),
    ]
    ab_in = nc.declare_dram_parameter("ab", [P, 2], F32, isOutput=False)
    # raw psum dumps: [0:128] prediction, [128:256] target
    out = nc.declare_dram_parameter("hist", [P, 256], F32, isOutput=True)
    out_jp = nc.declare_dram_parameter("jpad", [1, 2], I32, isOutput=True)

    with tile.TileContext(nc) as tc:
        with (
            tc.tile_pool(name="data", bufs=3) as dpool,
            tc.tile_pool(name="dig", bufs=3) as gpool,
            tc.tile_pool(name="oh", bufs=2) as ohpool,
            tc.tile_pool(name="const", bufs=1) as cpool,
            tc.tile_pool(name="psum", bufs=1, space="PSUM") as pp,
        ):
            ab_raw = cpool.tile([P, 2], F32)
            nc.sync.dma_start(ab_raw[:], ab_in[:])
            ab_a = cpool.tile([P, 2], F32)
            nc.scalar.copy(out=ab_a[:], in_=ab_raw[:])
            # consts: rh-extract scale/bias, Act-build -q biases, -1.0 scale
            c32 = cpool.tile([P, 2], F32)
            nc.vector.memset(c32[:, 0:1], 0.03125)
            nc.vector.memset(c32[:, 1:2], -0.484375)
            cneg = cpool.tile([P, N_ACT_PLAIN + 1], F32)
            for k in range(N_ACT_PLAIN):
                nc.vector.memset(cneg[:, k:k + 1], -float(k))
            nc.vector.memset(cneg[:, N_ACT_PLAIN:N_ACT_PLAIN + 1], -1.0)

            ps_p = pp.tile([P, 128], F32, tag="psP")
            ps_t = pp.tile([P, 128], F32, tag="psT")
            ps = [ps_p, ps_t]

            chunks = [(ti, ci) for ti in range(2) for ci in range(NCHUNK)]

            def phase_a(si):
                ti, ci = chunks[si]
                v = dpool.tile([P, C], F32, tag="v")
                nc.sync.dma_start(v[:], ins[ti][:, ci * C:(ci + 1) * C])
                ji = gpool.tile([P, C], I32, tag="ji")
                nc.scalar.activation(out=ji[:], in_=v[:], func=ACT.Identity,
                                     scale=ab_a[:, 0:1], bias=ab_a[:, 1:2])
                if ci == 0:
                    # partition 127 is all padding: export device pad bin
                    nc.sync.dma_start(out_jp[:, ti:ti + 1],
                                      ji[127:128, 0:1])
                rh32 = gpool.tile([P, C], I32, tag="rh32")
                nc.scalar.activation(out=rh32[:], in_=ji[:], func=ACT.Identity,
                                     scale=c32[:, 0:1], bias=c32[:, 1:2])
                m32 = gpool.tile([P, C], I32, tag="m32")
                nc.vector.tensor_scalar(out=m32[:], in0=ji[:], scalar1=31,
                                        scalar2=None, op0=ALU.bitwise_and)
                mb = gpool.tile([P, C], BF16, tag="mb")
                nc.scalar.copy(out=mb[:], in_=m32[:])
                rhb = gpool.tile([P, C], BF16, tag="rhb")
                nc.scalar.copy(out=rhb[:], in_=rh32[:])
                return mb, rhb

            def phase_b(si, mb, rhb):
                ti, ci = chunks[si]
                k_act = N_ACT_PLAIN if si % 2 == 0 else N_ACT_PLAIN - 1
                ohm = ohpool.tile([P, 32 * C], BF16, tag="ohm")
                ohr = ohpool.tile([P, 32 * C], BF16, tag="ohr")
                ohm4 = ohm[:].rearrange("p (cc q g) -> p cc q g", q=32, g=4)
                ohr4 = ohr[:].rearrange("p (cc q g) -> p cc q g", q=32, g=4)
                scratch = gpool.tile([P, C], BF16, tag="scratch")
                for q in range(32):
                    if q < k_act:
                        nc.scalar.activation(out=scratch[:], in_=mb[:],
                                             func=ACT.Square, scale=1.0,
                                             bias=cneg[:, q:q + 1])
                        nc.scalar.activation(
                            out=ohm4[:, :, q, :], in_=scratch[:],
                            func=ACT.Relu,
                            scale=cneg[:, N_ACT_PLAIN:N_ACT_PLAIN + 1],
                            bias=1.0)
                    else:
                        nc.vector.tensor_scalar(out=ohm4[:, :, q, :],
                                                in0=mb[:], scalar1=float(q),
                                                scalar2=None, op0=ALU.is_equal)
                for q in range(32):
                    nc.vector.tensor_scalar(out=ohr4[:, :, q, :], in0=rhb[:],
                                            scalar1=float(q), scalar2=None,
                                            op0=ALU.is_equal)
                for cc in range(NI):
                    nc.tensor.matmul(
                        ps[ti][:],
                        lhsT=ohm[:, cc * 128:(cc + 1) * 128],
                        rhs=ohr[:, cc * 128:(cc + 1) * 128],
                        start=(ci == 0 and cc == 0),
                        stop=(ci == NCHUNK - 1 and cc == NI - 1),
                    )
                if ci == NCHUNK - 1:
                    hsb = dpool.tile([P, 128], F32, tag="hsb")
                    nc.vector.tensor_copy(out=hsb[:], in_=ps[ti][:])
                    nc.sync.dma_start(out[:, ti * 128:(ti + 1) * 128], hsb[:])

            # software pipeline: A(si+1) emitted before B(si)
            cur = phase_a(0)
            for si in range(len(chunks)):
                nxt = phase_a(si + 1) if si + 1 < len(chunks) else None
                phase_b(si, *cur)
                cur = nxt
    nc.compile()
    return nc


_KERNELS = {}


def _get_kernels():
    if "mm" not in _KERNELS:
        _KERNELS["mm"] = _build_minmax()
        _KERNELS["hist"] = _build_hist()
    return _KERNELS["mm"], _KERNELS["hist"]


def _shard(flat):
    """Split [TOTAL] -> per-core padded [P, KTOT] tiles + pad values."""
    tiles, pads = [], []
    for c in range(NCORES):
        s = flat[c * SHARD:(c + 1) * SHARD]
        v0 = s[0]
        t = np.concatenate([s, np.full(PADN, v0, s.dtype)]).reshape(P, KTOT)
        tiles.append(t)
        pads.append(v0)
    return tiles, pads


def _psum_to_hist(X):
    """[P, 128] f32 psum dump -> [1024] f64 histogram.

    psum cell (m*4+g, rh*4+g') holds group-g counts on the g==g' diagonal."""
    Y = X.astype(np.float64).reshape(32, 4, 32, 4)   # [m, g, rh, g']
    diag = Y[:, np.arange(4), :, np.arange(4)]       # [g, m, rh]
    cnt = diag.sum(axis=0)                           # [m, rh]
    return cnt.T.ravel()                             # j = 32*rh + m


def kernel(prediction, target):
    nc_mm, nc_hist = _get_kernels()
    p = np.ascontiguousarray(np.asarray(prediction, dtype=np.float32).ravel())
    t = np.ascontiguousarray(np.asarray(target, dtype=np.float32).ravel())
    p_tiles, p_pads = _shard(p)
    t_tiles, t_pads = _shard(t)
    core_ids = list(range(NCORES))

    in_maps = [{"pv": p_tiles[c], "tv": t_tiles[c]} for c in core_ids]
    res = run_bass_kernel_spmd(nc_mm, in_maps, core_ids).results
    mm = np.stack([r["mm"][0] for r in res])        # [8, 2] = (-min, max)
    lo = np.float32(-(mm[:, 0].max()))
    hi = np.float32(mm[:, 1].max())

    dx = np.float32((hi - lo) / np.float32(NX - 1))
    A = np.float32(np.float32(1.0) / dx)
    B = np.float32(np.float32(-lo * A) + np.float32(0.5))
    ab = np.stack([np.full(P, A, np.float32), np.full(P, B, np.float32)],
                  axis=1)

    in_maps = [{"pv": p_tiles[c], "tv": t_tiles[c], "ab": ab}
               for c in core_ids]
    res = run_bass_kernel_spmd(nc_hist, in_maps, core_ids).results

    hp = np.zeros(1024, np.float64)
    ht = np.zeros(1024, np.float64)
    for c in core_ids:
        X = res[c]["hist"]                          # [P, 256] f32
        hp += _psum_to_hist(X[:, 0:128])
        ht += _psum_to_hist(X[:, 128:256])
        jp = res[c]["jpad"][0]                      # [2] i32, device pad bins
        hp[min(max(int(jp[0]), 0), 1023)] -= PADN
        ht[min(max(int(jp[1]), 0), 1023)] -= PADN

    hp[NX - 1] += hp[NX:].sum()
    ht[NX - 1] += ht[NX:].sum()
    cnt_p = np.cumsum(hp[:NX])
    cnt_t = np.cumsum(ht[:NX])

    n = np.float64(TOTAL)
    diff = np.abs(cnt_p / n - cnt_t / n)
    y = diff * diff
    x = np.linspace(np.float64(lo), np.float64(hi), NX)
    dxs = x[1:] - x[:-1]
    out = np.sum(0.5 * (y[1:] + y[:-1]) * dxs)
    return np.float32(out)


# revision 21
# speedup vs baseline: 1.0013x; 1.0013x over previous
"""CRPS loss kernel for Trainium2, 8 NeuronCores (SPMD data-parallel).

reference semantics:
    p, t = prediction.ravel(), target.ravel()       # N = 16,611,840 each
    lo, hi = min(min p, min t), max(max p, max t)
    x = linspace(lo, hi, 1000)  (f32)
    cdf_q(x_i) = #{v in q : v <= x_i} / N
    return trapz(|cdf_p - cdf_t|^2, x)

Device work (per core, 1/8 shard of each tensor):
  kernel A: min/max reduce (DVE X-reduces + Pool whole-tile max partials).
  kernel B: per element j = rint(v*A + B) in [0, 1000] (Act affine+round);
            digits m = j & 31 (DVE), rh = j >> 5 (Act scaled-round trick).
            Joint (m, rh) histogram via PACK4 block-diagonal PE matmuls:
            lhsT = one-hots of m (4 groups x 32 bins, column order m*4+g),
            rhs  = one-hots of rh (4 groups x 32 bins, order rh*4+g).
            PSUM accumulates the WHOLE tensor (counts < 2^24, exact f32);
            one psum->sbuf copy + DMA per tensor. The device's own binning
            of the pad value (partition 127 is all-pad) is exported as jpad
            so the host subtracts pads exactly.
Host: combine cores' [128, 256] f32 psum dumps -> exact 1024-bin histogram,
      fold j>=999, cumsum, 1000-point trapz in f64.
"""

import numpy as np
from concourse import bacc, mybir, tile
from concourse.bass_utils import run_bass_kernel_spmd

P = 128
NCORES = 8
TOTAL = 16 * 1 * 721 * 1440          # 16,611,840
SHARD = TOTAL // NCORES              # 2,076,480
KTOT = 16640                         # padded columns/core/tensor
PADN = P * KTOT - SHARD              # 53,440
NX = 1000
C = 640                              # chunk columns
NCHUNK = KTOT // C                   # 26 per tensor
NI = C // 4                          # PACK4 matmuls per chunk
RED = KTOT // 4                      # minmax reduce chunk

F32 = mybir.dt.float32
I32 = mybir.dt.int32
BF16 = mybir.dt.bfloat16
ALU = mybir.AluOpType
ACT = mybir.ActivationFunctionType

N_ACT_PLAIN = 6      # m-side bins built on Act (2-op square/relu)


def _build_minmax():
    nc = bacc.Bacc()
    ins = [
        nc.declare_dram_parameter("pv", [P, KTOT], F32, isOutput=False),
        nc.declare_dram_parameter("tv", [P, KTOT], F32, isOutput=False),
    ]
    out = nc.declare_dram_parameter("mm", [1, 2], F32, isOutput=True)

    with tile.TileContext(nc) as tc:
        with (
            tc.tile_pool(name="sbuf", bufs=6) as pool,
            tc.tile_pool(name="acc", bufs=1) as apool,
        ):
            ntile = 2 * (KTOT // RED)            # 8 tiles
            NPOOL = 5                            # tiles whose MAX runs on Pool
            mins = apool.tile([P, ntile], F32)
            maxs = apool.tile([P, ntile - NPOOL], F32)
            pmax1 = apool.tile([1, NPOOL], F32)
            t = 0
            for src in ins:
                for ci in range(KTOT // RED):
                    v = pool.tile([P, RED], F32, tag="v")
                    dmaeng = nc.sync if t % 2 == 0 else nc.scalar
                    dmaeng.dma_start(v[:], src[:, ci * RED:(ci + 1) * RED])
                    nc.vector.tensor_reduce(
                        mins[:, t:t + 1], v[:], mybir.AxisListType.X, ALU.min)
                    if t < NPOOL:
                        nc.gpsimd.tensor_reduce(
                            pmax1[:, t:t + 1], v[:], mybir.AxisListType.XYZWC,
                            ALU.max)
                    else:
                        nc.vector.tensor_reduce(
                            maxs[:, t - NPOOL:t - NPOOL + 1], v[:],
                            mybir.AxisListType.X, ALU.max)
                    t += 1
            pmin = apool.tile([P, 1], F32)
            pmax = apool.tile([P, 1], F32)
            nc.vector.tensor_reduce(pmin[:], mins[:], mybir.AxisListType.X,
                                    ALU.min)
            nc.vector.tensor_reduce(pmax[:], maxs[:], mybir.AxisListType.X,
                                    ALU.max)
            both = apool.tile([P, 2], F32)
            nc.vector.tensor_scalar(out=both[:, 0:1], in0=pmin[:], scalar1=-1.0,
                                    scalar2=None, op0=ALU.mult)
            nc.vector.tensor_copy(out=both[:, 1:2], in_=pmax[:])
            red = apool.tile([1, 2], F32)
            nc.gpsimd.tensor_reduce(red[:], both[:], mybir.AxisListType.C,
                                    ALU.max)
            pb = apool.tile([1, 1], F32)
            nc.vector.tensor_reduce(pb[:], pmax1[:], mybir.AxisListType.X,
                                    ALU.max)
            fin = apool.tile([1, 2], F32)
            nc.vector.tensor_copy(out=fin[:], in_=red[:])
            nc.vector.tensor_tensor(out=fin[:, 1:2], in0=red[:, 1:2],
                                    in1=pb[:], op=ALU.max)
            nc.sync.dma_start(out[:], fin[:])
    nc.compile()
    return nc


def _build_hist():
    nc = bacc.Bacc()
    ins = [
        nc.declare_dram_parameter("pv", [P, KTOT], F32, isOutput=False),
        nc.declare_dram_parameter("tv", [P, KTOT], F32, isOutput=False),
    ]
    ab_in = nc.declare_dram_parameter("ab", [P, 2], F32, isOutput=False)
    # raw psum dumps: [0:128] prediction, [128:256] target
    out = nc.declare_dram_parameter("hist", [P, 256], F32, isOutput=True)
    out_jp = nc.declare_dram_parameter("jpad", [1, 2], I32, isOutput=True)

    with tile.TileContext(nc) as tc:
        with (
            tc.tile_pool(name="data", bufs=3) as dpool,
            tc.tile_pool(name="dig", bufs=2) as gpool,
            tc.tile_pool(name="oh", bufs=2) as ohpool,
            tc.tile_pool(name="const", bufs=1) as cpool,
            tc.tile_pool(name="psum", bufs=1, space="PSUM") as pp,
        ):
            ab_raw = cpool.tile([P, 2], F32)
            nc.sync.dma_start(ab_raw[:], ab_in[:])
            ab_a = cpool.tile([P, 2], F32)
            nc.scalar.copy(out=ab_a[:], in_=ab_raw[:])
            # consts: rh-extract scale/bias, Act-build -q biases, -1.0 scale
            c32 = cpool.tile([P, 2], F32)
            nc.vector.memset(c32[:, 0:1], 0.03125)
            nc.vector.memset(c32[:, 1:2], -0.484375)
            cneg = cpool.tile([P, N_ACT_PLAIN + 1], F32)
            for k in range(N_ACT_PLAIN):
                nc.vector.memset(cneg[:, k:k + 1], -float(k))
            nc.vector.memset(cneg[:, N_ACT_PLAIN:N_ACT_PLAIN + 1], -1.0)

            ps_p = pp.tile([P, 128], F32, tag="psP")
            ps_t = pp.tile([P, 128], F32, tag="psT")
            ps = [ps_p, ps_t]

            chunks = [(ti, ci) for ti in range(2) for ci in range(NCHUNK)]

            def phase_a(si):
                ti, ci = chunks[si]
                v = dpool.tile([P, C], F32, tag="v")
                nc.sync.dma_start(v[:], ins[ti][:, ci * C:(ci + 1) * C])
                ji = gpool.tile([P, C], I32, tag="ji")
                nc.scalar.activation(out=ji[:], in_=v[:], func=ACT.Identity,
                                     scale=ab_a[:, 0:1], bias=ab_a[:, 1:2])
                if ci == 0:
                    # partition 127 is all padding: export device pad bin
                    nc.sync.dma_start(out_jp[:, ti:ti + 1],
                                      ji[127:128, 0:1])
                rh32 = gpool.tile([P, C], I32, tag="rh32")
                nc.scalar.activation(out=rh32[:], in_=ji[:], func=ACT.Identity,
                                     scale=c32[:, 0:1], bias=c32[:, 1:2])
                m32 = gpool.tile([P, C], I32, tag="m32")
                nc.vector.tensor_scalar(out=m32[:], in0=ji[:], scalar1=31,
                                        scalar2=None, op0=ALU.bitwise_and)
                mb = gpool.tile([P, C], BF16, tag="mb")
                nc.scalar.copy(out=mb[:], in_=m32[:])
                rhb = gpool.tile([P, C], BF16, tag="rhb")
                nc.scalar.copy(out=rhb[:], in_=rh32[:])
                return mb, rhb

            def phase_b(si, mb, rhb):
                ti, ci = chunks[si]
                ohm = ohpool.tile([P, 32 * C], BF16, tag="ohm")
                ohr = ohpool.tile([P, 32 * C], BF16, tag="ohr")
                ohm4 = ohm[:].rearrange("p (cc q g) -> p cc q g", q=32, g=4)
                ohr4 = ohr[:].rearrange("p (cc q g) -> p cc q g", q=32, g=4)
                scratch = gpool.tile([P, C], BF16, tag="scratch")
                for q in range(32):
                    if q < N_ACT_PLAIN:
                        nc.scalar.activation(out=scratch[:], in_=mb[:],
                                             func=ACT.Square, scale=1.0,
                                             bias=cneg[:, q:q + 1])
                        nc.scalar.activation(
                            out=ohm4[:, :, q, :], in_=scratch[:],
                            func=ACT.Relu,
                            scale=cneg[:, N_ACT_PLAIN:N_ACT_PLAIN + 1],
                            bias=1.0)
                    else:
                        nc.vector.tensor_scalar(out=ohm4[:, :, q, :],
                                                in0=mb[:], scalar1=float(q),
                                                scalar2=None, op0=ALU.is_equal)
                for q in range(32):
                    nc.vector.tensor_scalar(out=ohr4[:, :, q, :], in0=rhb[:],
                                            scalar1=float(q), scalar2=None,
                                            op0=ALU.is_equal)
                for cc in range(NI):
                    nc.tensor.matmul(
                        ps[ti][:],
                        lhsT=ohm[:, cc * 128:(cc + 1) * 128],
                        rhs=ohr[:, cc * 128:(cc + 1) * 128],
                        start=(ci == 0 and cc == 0),
                        stop=(ci == NCHUNK - 1 and cc == NI - 1),
                    )
                if ci == NCHUNK - 1:
                    hsb = dpool.tile([P, 128], F32, tag="hsb")
                    nc.vector.tensor_copy(out=hsb[:], in_=ps[ti][:])
                    nc.sync.dma_start(out[:, ti * 128:(ti + 1) * 128], hsb[:])

            # software pipeline: A(si+1) emitted before B(si)
            cur = phase_a(0)
            for si in range(len(chunks)):
                nxt = phase_a(si + 1) if si + 1 < len(chunks) else None
                phase_b(si, *cur)
                cur = nxt
    nc.compile()
    return nc


_KERNELS = {}


def _get_kernels():
    if "mm" not in _KERNELS:
        _KERNELS["mm"] = _build_minmax()
        _KERNELS["hist"] = _build_hist()
    return _KERNELS["mm"], _KERNELS["hist"]


def _shard(flat):
    """Split [TOTAL] -> per-core padded [P, KTOT] tiles + pad values."""
    tiles, pads = [], []
    for c in range(NCORES):
        s = flat[c * SHARD:(c + 1) * SHARD]
        v0 = s[0]
        t = np.concatenate([s, np.full(PADN, v0, s.dtype)]).reshape(P, KTOT)
        tiles.append(t)
        pads.append(v0)
    return tiles, pads


def _psum_to_hist(X):
    """[P, 128] f32 psum dump -> [1024] f64 histogram.

    psum cell (m*4+g, rh*4+g') holds group-g counts on the g==g' diagonal."""
    Y = X.astype(np.float64).reshape(32, 4, 32, 4)   # [m, g, rh, g']
    diag = Y[:, np.arange(4), :, np.arange(4)]       # [g, m, rh]
    cnt = diag.sum(axis=0)                           # [m, rh]
    return cnt.T.ravel()                             # j = 32*rh + m


def kernel(prediction, target):
    nc_mm, nc_hist = _get_kernels()
    p = np.ascontiguousarray(np.asarray(prediction, dtype=np.float32).ravel())
    t = np.ascontiguousarray(np.asarray(target, dtype=np.float32).ravel())
    p_tiles, p_pads = _shard(p)
    t_tiles, t_pads = _shard(t)
    core_ids = list(range(NCORES))

    in_maps = [{"pv": p_tiles[c], "tv": t_tiles[c]} for c in core_ids]
    res = run_bass_kernel_spmd(nc_mm, in_maps, core_ids).results
    mm = np.stack([r["mm"][0] for r in res])        # [8, 2] = (-min, max)
    lo = np.float32(-(mm[:, 0].max()))
    hi = np.float32(mm[:, 1].max())

    dx = np.float32((hi - lo) / np.float32(NX - 1))
    A = np.float32(np.float32(1.0) / dx)
    B = np.float32(np.float32(-lo * A) + np.float32(0.5))
    ab = np.stack([np.full(P, A, np.float32), np.full(P, B, np.float32)],
                  axis=1)

    in_maps = [{"pv": p_tiles[c], "tv": t_tiles[c], "ab": ab}
               for c in core_ids]
    res = run_bass_kernel_spmd(nc_hist, in_maps, core_ids).results

    hp = np.zeros(1024, np.float64)
    ht = np.zeros(1024, np.float64)
    for c in core_ids:
        X = res[c]["hist"]                          # [P, 256] f32
        hp += _psum_to_hist(X[:, 0:128])
        ht += _psum_to_hist(X[:, 128:256])
        jp = res[c]["jpad"][0]                      # [2] i32, device pad bins
        hp[min(max(int(jp[0]), 0), 1023)] -= PADN
        ht[min(max(int(jp[1]), 0), 1023)] -= PADN

    hp[NX - 1] += hp[NX:].sum()
    ht[NX - 1] += ht[NX:].sum()
    cnt_p = np.cumsum(hp[:NX])
    cnt_t = np.cumsum(ht[:NX])

    n = np.float64(TOTAL)
    diff = np.abs(cnt_p / n - cnt_t / n)
    y = diff * diff
    x = np.linspace(np.float64(lo), np.float64(hi), NX)
    dxs = x[1:] - x[:-1]
    out = np.sum(0.5 * (y[1:] + y[:-1]) * dxs)
    return np.float32(out)
